# revision 23
# baseline (speedup 1.0000x reference)
"""RWKV WKV attention block on 8 Trainium2 NeuronCores.

Sharding: data-parallel over batch B=8 -> one batch element per core.
Per-core pipeline (T=2048 in chunks of Tc=256):
  x^T (pre-transposed, bf16) -> dx-form time-mix (dx on DVE, then
  tm*dx on ACT + add-shifted on Pool: one pair per mix tensor) ->
  K/V/R projections (PE, bf16, fp32 PSUM accumulate) -> exp/sigmoid
  (ACT, PSUM read) -> per-group num/den WKV recurrences as DVE
  tensor_tensor_scan ops chained across chunks via `initial` APs (no
  carry-copy barrier; the multiplier tile has 1.0 in column 0 so the
  carry lands in the output for the shifted numer/denom reads) ->
  numer/denom STT + reciprocal on DVE, gate muls on Pool -> output
  projection (PE) software-pipelined one chunk behind so the PE never
  waits on the gate tail -> natural-layout DMA out.
All weights are pre-transposed on the host so the contraction dim lands on
partitions for every matmul. ISA notes honored here: scalar_tensor_tensor
and per-partition scalars are DVE/ACT-only, GPSIMD cannot touch PSUM.
"""

import sys

for _p in ("/opt/trn_rl_repo", "/root/.axon_site/_ro/trn_rl_repo"):
    if _p not in sys.path:
        sys.path.append(_p)

import numpy as np

import concourse.bass as bass
import concourse.mybir as mybir
import concourse.tile as tile
from concourse.bass_utils import run_bass_kernel_spmd

F32 = mybir.dt.float32
BF16 = mybir.dt.bfloat16
FP8 = mybir.dt.float8e4
MMDT = BF16
ALU = mybir.AluOpType
ACT_F = mybir.ActivationFunctionType
PM = mybir.MatmulPerfMode

# The R projection runs in fp8e4 DoubleRow mode (2x PE throughput). Wr is
# pre-scaled by RSCALE on the host so its entries sit in e4m3's normal
# range (38% would be subnormal unscaled); the sigmoid folds 1/RSCALE back
# in via its scale operand. Measured end-to-end rel_fro: 1.37e-2 vs the
# 2e-2 gate (bf16 everywhere: 4.8e-3).
RSCALE = 32.0

B, T, D = 8, 2048, 1024
P = 128
G = D // P          # 8 channel groups
TC = 256            # T chunk
NCH = T // TC       # 8 chunks


def _split_waits(nc, maxw=1):
    """walrus in this image rejects >1 sync-wait per instruction; move the
    excess onto preceding same-engine no-ops (semantically identical)."""
    for f in nc.m.functions:
        for bb in f.blocks:
            new_insts = []
            for ins in bb.instructions:
                si = ins.sync_info
                if si is not None and si.on_wait and len(si.on_wait) > maxw:
                    waits = list(si.on_wait)
                    extra, keep = waits[:-maxw], waits[-maxw:]
                    for i in range(0, len(extra), maxw):
                        nop = mybir.InstNoOp(name=f"{ins.name}-ws{i}", ins=[], outs=[])
                        nop.engine = ins.engine
                        nop.sync_info = mybir.SyncInfo(
                            on_wait=extra[i:i + maxw], on_update=[])
                        new_insts.append(nop)
                        nc.register_instruction(nop, overwrite=True)
                    si.on_wait = keep
                new_insts.append(ins)
            bb.instructions = new_insts


def _build_nc(reps=None):
    nc = bass.Bass()

    xT = nc.declare_dram_parameter("xT", [P, G, T + 1], BF16, isOutput=False)
    WkT = nc.declare_dram_parameter("WkT", [P, G, D], MMDT, isOutput=False)
    WvT = nc.declare_dram_parameter("WvT", [P, G, D], MMDT, isOutput=False)
    WrT = nc.declare_dram_parameter("WrT", [P, G, D], FP8, isOutput=False)
    WoT = nc.declare_dram_parameter("WoT", [P, G, D], MMDT, isOutput=False)
    ew_p = nc.declare_dram_parameter("ew", [P, G], F32, isOutput=False)
    etf_p = nc.declare_dram_parameter("etf", [P, G], F32, isOutput=False)
    tmk_p = nc.declare_dram_parameter("tmk", [P, G], F32, isOutput=False)
    tmv_p = nc.declare_dram_parameter("tmv", [P, G], F32, isOutput=False)
    tmr_p = nc.declare_dram_parameter("tmr", [P, G], F32, isOutput=False)
    ln_p = nc.declare_dram_parameter("lnum", [P, G], F32, isOutput=False)
    ld_p = nc.declare_dram_parameter("lden", [P, G], F32, isOutput=False)
    out_p = nc.declare_dram_parameter("out", [T, D], F32, isOutput=True)

    with tile.TileContext(nc) as tc:
        with tc.tile_pool(name="wts", bufs=1) as wts, \
             tc.tile_pool(name="consts", bufs=1) as consts, \
             tc.tile_pool(name="xs", bufs=2) as xs, \
             tc.tile_pool(name="mix", bufs=2) as mixp, \
             tc.tile_pool(name="dxp", bufs=2) as dxp, \
             tc.tile_pool(name="t1p", bufs=3) as t1p, \
             tc.tile_pool(name="ekb", bufs=1) as ekb, \
             tc.tile_pool(name="scano", bufs=2) as scano, \
             tc.tile_pool(name="work", bufs=3) as work, \
             tc.tile_pool(name="wsrp", bufs=2) as wsrp, \
             tc.tile_pool(name="wop", bufs=1) as wop, \
             tc.tile_pool(name="outp", bufs=4) as outp, \
             tc.tile_pool(name="pskvr", bufs=2, space="PSUM") as pskvr, \
             tc.tile_pool(name="psout", bufs=2, space="PSUM") as psout:

            def emit_all():
                # DMA emission order matters for startup latency: chunk-0 x and
                # the small consts first, then Wk (first weights the PE needs),
                # then Wv/Wr, then Wo (only needed at end of chunk 0).
                def load_xw(c):
                    t = xs.tile([P, G, TC + 1], BF16, tag="xw")
                    nc.sync.dma_start(t[:], xT[:, :, c * TC:c * TC + TC + 1])
                    return t

                xw_next = load_xw(0)

                ew_sb = consts.tile([P, G], F32, tag="ew")
                etf_sb = consts.tile([P, G], F32, tag="etf")
                tmk_sb = consts.tile([P, G], F32, tag="tmk")
                tmv_sb = consts.tile([P, G], F32, tag="tmv")
                tmr_sb = consts.tile([P, G], F32, tag="tmr")
                ln_sb = consts.tile([P, G], F32, tag="ln")
                ld_sb = consts.tile([P, G], F32, tag="ld")
                nc.sync.dma_start(tmk_sb[:], tmk_p[:])
                nc.sync.dma_start(tmv_sb[:], tmv_p[:])
                nc.sync.dma_start(tmr_sb[:], tmr_p[:])
                nc.sync.dma_start(ew_sb[:], ew_p[:])
                nc.sync.dma_start(etf_sb[:], etf_p[:])
                nc.sync.dma_start(ln_sb[:], ln_p[:])
                nc.sync.dma_start(ld_sb[:], ld_p[:])

                wk = wts.tile([P, G, D], MMDT, tag="wk")
                wv = wts.tile([P, G, D], MMDT, tag="wv")
                wr = wts.tile([P, G, D], FP8, tag="wr")
                for ig in range(G):
                    nc.sync.dma_start(wk[:, ig], WkT[:, ig])
                for ig in range(G):
                    nc.sync.dma_start(wv[:, ig], WvT[:, ig])
                for ig in range(G):
                    nc.sync.dma_start(wr[:, ig], WrT[:, ig])

                # broadcast time-mix tiles so the mixes can run on Pool
                # (per-partition scalars are DVE/ACT-only on this ISA).
                tmkf = consts.tile([P, G, TC], BF16, tag="tmkf")
                tmvf = consts.tile([P, G, TC], BF16, tag="tmvf")
                tmrf = consts.tile([P, G, TC], BF16, tag="tmrf")
                for tmf, tm_sb in ((tmkf, tmk_sb), (tmvf, tmv_sb), (tmrf, tmr_sb)):
                    for ig in range(G):
                        nc.vector.tensor_copy(
                            tmf[:, ig, :],
                            tm_sb[:, ig:ig + 1].to_broadcast([P, TC]))

                # scan multiplier const: col 0 = 1.0 (passes the carry coming
                # in via `initial` straight to out[0]), cols 1..TC = ew.
                scanmul = consts.tile([P, G, TC + 1], F32, tag="scanmul")
                for jg in range(G):
                    nc.vector.tensor_copy(
                        scanmul[:, jg, :],
                        ew_sb[:, jg:jg + 1].to_broadcast([P, TC + 1]))
                nc.vector.memset(scanmul[:, :, 0], 1.0)

                # scan input streams [P, G, 1+TC]; col 0 stays 0 forever so
                # out[0] = 1.0*initial + 0 = carry.
                ekvbuf = ekb.tile([P, G, TC + 1], F32, tag="ekv")
                ekbuf = ekb.tile([P, G, TC + 1], F32, tag="ek")
                nc.vector.memset(ekvbuf[:, :, 0], 0.0)
                nc.vector.memset(ekbuf[:, :, 0], 0.0)

                # Wo resident (loaded last; first needed at end of chunk 0)
                wo = wop.tile([P, G, D], MMDT, tag="wo")
                for ig in range(G):
                    nc.sync.dma_start(wo[:, ig], WoT[:, ig])

                def emit_opass(wsr, t0):
                    for dt in range(2):
                        for ts in range(2):
                            ops = psout.tile([P, 512], F32, tag="ops")
                            for jg in range(G):
                                nc.tensor.matmul(
                                    ops[:], wsr[:, jg, bass.ts(ts, P)],
                                    wo[:, jg, bass.ts(dt, 512)],
                                    start=(jg == 0), stop=(jg == G - 1))
                            ob = outp.tile([P, 512], F32, tag="ob")
                            nc.scalar.copy(ob[:], ops[:])
                            nc.sync.dma_start(
                                out_p[t0 + ts * P:t0 + (ts + 1) * P,
                                      bass.ts(dt, 512)], ob[:])

                wsr_prev = None
                t0_prev = 0
                numout_prev = None
                denout_prev = None
                for c in range(NCH):
                    t0 = c * TC

                    # x^T window [P, G, TC+1]: col 0 is t0-1 (or last_x for c=0)
                    xw = xw_next
                    if c + 1 < NCH:
                        xw_next = load_xw(c + 1)

                    # dx-form time-mix: dx = cur - sh (DVE, bf16 2x mode);
                    # mix = tm*dx (Pool mul vs broadcast tm tile) + sh (Pool).
                    # xr additionally gets a DVE cast into fp8 pair tiles
                    # ([P, 2, TC]: two k-tiles stacked for DoubleRow).
                    xk, xv, xr8 = [], [], []
                    for ig2 in range(G // 2):
                        t8 = dxp.tile([P, 2, TC], FP8, tag=f"xr8{ig2}",
                                      name=f"xr8_{ig2}")
                        xr8.append(t8)
                    for ig in range(G):
                        cur = xw[:, ig, 1:1 + TC]
                        sh = xw[:, ig, 0:TC]
                        dx = dxp.tile([P, TC], BF16, tag=f"dx{ig}")
                        nc.gpsimd.tensor_sub(dx[:], cur, sh)
                        for nm, tmf, lst in (("xk", tmkf, xk),
                                             ("xv", tmvf, xv),
                                             ("xr", tmrf, None)):
                            t1 = t1p.tile([P, TC], BF16, tag=f"t1{nm}")
                            nc.gpsimd.tensor_mul(t1[:], dx[:], tmf[:, ig, :])
                            t = mixp.tile([P, TC], MMDT, tag=f"{nm}{ig}")
                            nc.gpsimd.tensor_add(t[:], t1[:], sh)
                            if lst is None:
                                # quantize the r-mix to fp8 on ACT (DVE is
                                # the nearest competitor to the PE now)
                                nc.scalar.copy(xr8[ig // 2][:, ig % 2, :], t[:])
                            else:
                                lst.append(t)

                    numout = scano.tile([P, G, TC + 1], F32, tag="numout")
                    denout = scano.tile([P, G, TC + 1], F32, tag="denout")

                    # ---- pass KV: k & v projections + WKV scans ----
                    wsr = wsrp.tile([P, G, TC], MMDT, tag="wsr")
                    for jg in range(G):
                        jsl = bass.ts(jg, P)
                        kps = pskvr.tile([P, TC], F32, tag="kps")
                        for ig in range(G):
                            nc.tensor.matmul(kps[:], wk[:, ig, jsl], xk[ig][:],
                                             start=(ig == 0), stop=(ig == G - 1))
                        vps = pskvr.tile([P, TC], F32, tag="vps")
                        for ig in range(G):
                            nc.tensor.matmul(vps[:], wv[:, ig, jsl], xv[ig][:],
                                             start=(ig == 0), stop=(ig == G - 1))
                        ekap = ekbuf[:, jg, 1:TC + 1]
                        nc.scalar.activation(ekap, kps[:], ACT_F.Exp)
                        nc.vector.tensor_mul(ekvbuf[:, jg, 1:TC + 1], ekap, vps[:])

                        if c == 0:
                            init_n = ln_sb[:, jg:jg + 1]
                            init_d = ld_sb[:, jg:jg + 1]
                        else:
                            init_n = numout_prev[:, jg, TC:TC + 1]
                            init_d = denout_prev[:, jg, TC:TC + 1]
                        nc.vector.tensor_tensor_scan(
                            numout[:, jg, :], scanmul[:, jg, :], ekvbuf[:, jg, :],
                            init_n, ALU.mult, ALU.add)
                        nc.vector.tensor_tensor_scan(
                            denout[:, jg, :], scanmul[:, jg, :], ekbuf[:, jg, :],
                            init_d, ALU.mult, ALU.add)

                        # euk = ek*etf, so:
                        # numer = num_{t-1} + etf*ekv ; denom = den_{t-1} + etf*ek
                        numer = work.tile([P, TC], F32, tag="numer")
                        denom = work.tile([P, TC], F32, tag="denom")
                        etfs = etf_sb[:, jg:jg + 1]
                        nc.vector.scalar_tensor_tensor(
                            numer[:], ekvbuf[:, jg, 1:TC + 1], etfs,
                            numout[:, jg, 0:TC], ALU.mult, ALU.add)
                        nc.vector.scalar_tensor_tensor(
                            denom[:], ekap, etfs,
                            denout[:, jg, 0:TC], ALU.mult, ALU.add)
                        rden = work.tile([P, TC], F32, tag="rden")
                        nc.vector.reciprocal(rden[:], denom[:])
                        nc.gpsimd.tensor_mul(wsr[:, jg], numer[:], rden[:])

                    numout_prev, denout_prev = numout, denout

                    # ---- pass R: fp8 DoubleRow r projections (2 k-tiles per
                    # matmul, 2x PE rate), sigmoid folds the 1/RSCALE back in,
                    # then wsr *= sigmoid(r) ----
                    for jg in range(G):
                        jsl = bass.ts(jg, P)
                        rps = pskvr.tile([P, TC], F32, tag="rps")
                        for ig2 in range(G // 2):
                            nc.tensor.matmul(
                                rps[:], wr[:, 2 * ig2:2 * ig2 + 2, jsl],
                                xr8[ig2][:], start=(ig2 == 0),
                                stop=(ig2 == G // 2 - 1), perf_mode=PM.DoubleRow)
                        sr = work.tile([P, TC], F32, tag="sr")
                        nc.scalar.activation(sr[:], rps[:], ACT_F.Sigmoid,
                                             scale=1.0 / RSCALE)
                        nc.gpsimd.tensor_mul(wsr[:, jg], wsr[:, jg], sr[:])

                    # ---- pass O for the PREVIOUS chunk (software pipeline:
                    # wsr of c-1 is long since ready, so the PE never stalls
                    # on this chunk's gate tail) ----
                    if wsr_prev is not None:
                        emit_opass(wsr_prev, t0_prev)
                    wsr_prev, t0_prev = wsr, t0

                emit_opass(wsr_prev, t0_prev)

            for _ in range(reps or 1):
                emit_all()

    _split_waits(nc, 1)
    return nc


_NC_CACHE = None


def _get_nc():
    global _NC_CACHE
    if _NC_CACHE is None:
        _NC_CACHE = _build_nc()
    return _NC_CACHE


def _pg(v):
    """(D,) channel vector -> [P, G] with channel d = g*128 + p."""
    return np.ascontiguousarray(np.asarray(v, np.float32).reshape(G, P).T)


def _wt(w, dt=None, scale=1.0):
    """W (D_out, D_in) -> W.T tiled [P, G, D_out] (contraction on partitions)."""
    wt = np.asarray(w, np.float32).T * scale  # (D_in, D_out)
    out = np.ascontiguousarray(wt.reshape(G, P, D).transpose(1, 0, 2))
    return out.astype(mybir.dt.np(dt or MMDT))


def kernel(x, last_x, last_num, last_den, time_decay, time_first,
           time_mix_k, time_mix_v, time_mix_r, Wk, Wv, Wr, Wo):
    x = np.asarray(x, np.float32)
    last_x = np.asarray(last_x, np.float32)
    last_num = np.asarray(last_num, np.float32)
    last_den = np.asarray(last_den, np.float32)

    ew = _pg(np.exp(-np.exp(np.asarray(time_decay, np.float64))))
    etf = _pg(np.exp(np.asarray(time_first, np.float64)))
    tmk = _pg(np.asarray(time_mix_k).reshape(-1))
    tmv = _pg(np.asarray(time_mix_v).reshape(-1))
    tmr = _pg(np.asarray(time_mix_r).reshape(-1))
    wkT, wvT, woT = _wt(Wk), _wt(Wv), _wt(Wo)
    wrT = _wt(Wr, dt=FP8, scale=RSCALE)
    bf = mybir.dt.np(BF16)

    in_maps = []
    for b in range(B):
        xs = np.concatenate([last_x[b], x[b]], axis=0)      # (T+1, D)
        xTb = np.ascontiguousarray(
            xs.T.reshape(G, P, T + 1).transpose(1, 0, 2)).astype(bf)
        in_maps.append({
            "xT": xTb,
            "WkT": wkT, "WvT": wvT, "WrT": wrT, "WoT": woT,
            "ew": ew, "etf": etf, "tmk": tmk, "tmv": tmv, "tmr": tmr,
            "lnum": _pg(last_num[b, 0]), "lden": _pg(last_den[b, 0]),
        })

    global _last_in_maps
    _last_in_maps = in_maps
    nc = _get_nc()
    res = run_bass_kernel_spmd(nc, in_maps, list(range(B)))
    return np.stack([res.results[b]["out"] for b in range(B)], axis=0)


_last_in_maps = None


# revision 24
# speedup vs baseline: 1.5984x; 1.5984x over previous
"""RWKV WKV attention block on 8 Trainium2 NeuronCores.

Sharding: data-parallel over batch B=8 -> one batch element per core.
Per-core pipeline (T=2048 in chunks of Tc=256):
  x^T (pre-transposed, bf16) -> dx-form time-mix (dx on DVE, then
  tm*dx on ACT + add-shifted on Pool: one pair per mix tensor) ->
  K/V/R projections (PE, bf16, fp32 PSUM accumulate) -> exp/sigmoid
  (ACT, PSUM read) -> per-group num/den WKV recurrences as DVE
  tensor_tensor_scan ops chained across chunks via `initial` APs (no
  carry-copy barrier; the multiplier tile has 1.0 in column 0 so the
  carry lands in the output for the shifted numer/denom reads) ->
  numer/denom STT + reciprocal on DVE, gate muls on Pool -> output
  projection (PE) software-pipelined one chunk behind so the PE never
  waits on the gate tail -> natural-layout DMA out.
All weights are pre-transposed on the host so the contraction dim lands on
partitions for every matmul. ISA notes honored here: scalar_tensor_tensor
and per-partition scalars are DVE/ACT-only, GPSIMD cannot touch PSUM.
"""

import sys

for _p in ("/opt/trn_rl_repo", "/root/.axon_site/_ro/trn_rl_repo"):
    if _p not in sys.path:
        sys.path.append(_p)

import numpy as np

import concourse.bass as bass
import concourse.mybir as mybir
import concourse.tile as tile
from concourse.bass_utils import run_bass_kernel_spmd

F32 = mybir.dt.float32
BF16 = mybir.dt.bfloat16
FP8 = mybir.dt.float8e4
MMDT = BF16
ALU = mybir.AluOpType
ACT_F = mybir.ActivationFunctionType
PM = mybir.MatmulPerfMode

# The R projection runs in fp8e4 DoubleRow mode (2x PE throughput). Wr is
# pre-scaled by RSCALE on the host so its entries sit in e4m3's normal
# range (38% would be subnormal unscaled); the sigmoid folds 1/RSCALE back
# in via its scale operand. Measured end-to-end rel_fro: 1.37e-2 vs the
# 2e-2 gate (bf16 everywhere: 4.8e-3).
RSCALE = 32.0

B, T, D = 8, 2048, 1024
P = 128
G = D // P          # 8 channel groups
TC = 256            # T chunk
NCH = T // TC       # 8 chunks


def _split_waits(nc, maxw=1):
    """walrus in this image rejects >1 sync-wait per instruction; move the
    excess onto preceding same-engine no-ops (semantically identical)."""
    for f in nc.m.functions:
        for bb in f.blocks:
            new_insts = []
            for ins in bb.instructions:
                si = ins.sync_info
                if si is not None and si.on_wait and len(si.on_wait) > maxw:
                    waits = list(si.on_wait)
                    extra, keep = waits[:-maxw], waits[-maxw:]
                    for i in range(0, len(extra), maxw):
                        nop = mybir.InstNoOp(name=f"{ins.name}-ws{i}", ins=[], outs=[])
                        nop.engine = ins.engine
                        nop.sync_info = mybir.SyncInfo(
                            on_wait=extra[i:i + maxw], on_update=[])
                        new_insts.append(nop)
                        nc.register_instruction(nop, overwrite=True)
                    si.on_wait = keep
                new_insts.append(ins)
            bb.instructions = new_insts


def _build_nc(reps=None):
    nc = bass.Bass()

    xT = nc.declare_dram_parameter("xT", [P, G, T + 1], BF16, isOutput=False)
    WkT = nc.declare_dram_parameter("WkT", [P, G, D], MMDT, isOutput=False)
    WvT = nc.declare_dram_parameter("WvT", [P, G, D], MMDT, isOutput=False)
    WrT = nc.declare_dram_parameter("WrT", [P, G, D], FP8, isOutput=False)
    WoT = nc.declare_dram_parameter("WoT", [P, G, D], MMDT, isOutput=False)
    ew_p = nc.declare_dram_parameter("ew", [P, G], F32, isOutput=False)
    etf_p = nc.declare_dram_parameter("etf", [P, G], F32, isOutput=False)
    tmk_p = nc.declare_dram_parameter("tmk", [P, G], F32, isOutput=False)
    tmv_p = nc.declare_dram_parameter("tmv", [P, G], F32, isOutput=False)
    tmr_p = nc.declare_dram_parameter("tmr", [P, G], F32, isOutput=False)
    ln_p = nc.declare_dram_parameter("lnum", [P, G], F32, isOutput=False)
    ld_p = nc.declare_dram_parameter("lden", [P, G], F32, isOutput=False)
    out_p = nc.declare_dram_parameter("out", [T, D], F32, isOutput=True)

    with tile.TileContext(nc) as tc:
        with tc.tile_pool(name="wts", bufs=1) as wts, \
             tc.tile_pool(name="consts", bufs=1) as consts, \
             tc.tile_pool(name="xs", bufs=2) as xs, \
             tc.tile_pool(name="mix", bufs=2) as mixp, \
             tc.tile_pool(name="dxp", bufs=2) as dxp, \
             tc.tile_pool(name="t1p", bufs=3) as t1p, \
             tc.tile_pool(name="ekb", bufs=1) as ekb, \
             tc.tile_pool(name="scano", bufs=2) as scano, \
             tc.tile_pool(name="work", bufs=3) as work, \
             tc.tile_pool(name="wsrp", bufs=2) as wsrp, \
             tc.tile_pool(name="wop", bufs=1) as wop, \
             tc.tile_pool(name="outp", bufs=4) as outp, \
             tc.tile_pool(name="pskvr", bufs=2, space="PSUM") as pskvr, \
             tc.tile_pool(name="psout", bufs=2, space="PSUM") as psout:

            def emit_all():
                # DMA emission order matters for startup latency: chunk-0 x and
                # the small consts first, then Wk (first weights the PE needs),
                # then Wv/Wr, then Wo (only needed at end of chunk 0).
                def load_xw(c):
                    t = xs.tile([P, G, TC + 1], BF16, tag="xw")
                    nc.sync.dma_start(t[:], xT[:, :, c * TC:c * TC + TC + 1])
                    return t

                xw_next = load_xw(0)

                ew_sb = consts.tile([P, G], F32, tag="ew")
                etf_sb = consts.tile([P, G], F32, tag="etf")
                tmk_sb = consts.tile([P, G], F32, tag="tmk")
                tmv_sb = consts.tile([P, G], F32, tag="tmv")
                tmr_sb = consts.tile([P, G], F32, tag="tmr")
                ln_sb = consts.tile([P, G], F32, tag="ln")
                ld_sb = consts.tile([P, G], F32, tag="ld")
                nc.sync.dma_start(tmk_sb[:], tmk_p[:])
                nc.sync.dma_start(tmv_sb[:], tmv_p[:])
                nc.sync.dma_start(tmr_sb[:], tmr_p[:])
                nc.sync.dma_start(ew_sb[:], ew_p[:])
                nc.sync.dma_start(etf_sb[:], etf_p[:])
                nc.sync.dma_start(ln_sb[:], ln_p[:])
                nc.sync.dma_start(ld_sb[:], ld_p[:])

                wk = wts.tile([P, G, D], MMDT, tag="wk")
                wv = wts.tile([P, G, D], MMDT, tag="wv")
                wr = wts.tile([P, G, D], FP8, tag="wr")
                for ig in range(G):
                    nc.sync.dma_start(wk[:, ig], WkT[:, ig])
                for ig in range(G):
                    nc.sync.dma_start(wv[:, ig], WvT[:, ig])
                for ig in range(G):
                    nc.sync.dma_start(wr[:, ig], WrT[:, ig])

                # broadcast time-mix tiles so the mixes can run on Pool
                # (per-partition scalars are DVE/ACT-only on this ISA).
                tmkf = consts.tile([P, G, TC], BF16, tag="tmkf")
                tmvf = consts.tile([P, G, TC], BF16, tag="tmvf")
                tmrf = consts.tile([P, G, TC], BF16, tag="tmrf")
                for tmf, tm_sb in ((tmkf, tmk_sb), (tmvf, tmv_sb), (tmrf, tmr_sb)):
                    for ig in range(G):
                        nc.vector.tensor_copy(
                            tmf[:, ig, :],
                            tm_sb[:, ig:ig + 1].to_broadcast([P, TC]))

                # scan multiplier const: col 0 = 1.0 (passes the carry coming
                # in via `initial` straight to out[0]), cols 1..TC = ew.
                scanmul = consts.tile([P, G, TC + 1], F32, tag="scanmul")
                for jg in range(G):
                    nc.vector.tensor_copy(
                        scanmul[:, jg, :],
                        ew_sb[:, jg:jg + 1].to_broadcast([P, TC + 1]))
                nc.vector.memset(scanmul[:, :, 0], 1.0)

                # scan input streams [P, G, 1+TC]; col 0 stays 0 forever so
                # out[0] = 1.0*initial + 0 = carry.
                ekvbuf = ekb.tile([P, G, TC + 1], F32, tag="ekv")
                ekbuf = ekb.tile([P, G, TC + 1], F32, tag="ek")
                nc.vector.memset(ekvbuf[:, :, 0], 0.0)
                nc.vector.memset(ekbuf[:, :, 0], 0.0)

                # Wo resident (loaded last; first needed at end of chunk 0)
                wo = wop.tile([P, G, D], MMDT, tag="wo")
                for ig in range(G):
                    nc.sync.dma_start(wo[:, ig], WoT[:, ig])

                def emit_opass(wsr, t0):
                    for dt in range(2):
                        for ts in range(2):
                            ops = psout.tile([P, 512], F32, tag="ops")
                            for jg in range(G):
                                nc.tensor.matmul(
                                    ops[:], wsr[:, jg, bass.ts(ts, P)],
                                    wo[:, jg, bass.ts(dt, 512)],
                                    start=(jg == 0), stop=(jg == G - 1))
                            ob = outp.tile([P, 512], F32, tag="ob")
                            nc.scalar.copy(ob[:], ops[:])
                            nc.sync.dma_start(
                                out_p[t0 + ts * P:t0 + (ts + 1) * P,
                                      bass.ts(dt, 512)], ob[:])

                wsr_prev = None
                t0_prev = 0
                numout_prev = None
                denout_prev = None
                for c in range(NCH):
                    t0 = c * TC

                    # x^T window [P, G, TC+1]: col 0 is t0-1 (or last_x for c=0)
                    xw = xw_next
                    if c + 1 < NCH:
                        xw_next = load_xw(c + 1)

                    # dx-form time-mix: dx = cur - sh (DVE, bf16 2x mode);
                    # mix = tm*dx (Pool mul vs broadcast tm tile) + sh (Pool).
                    # xr additionally gets a DVE cast into fp8 pair tiles
                    # ([P, 2, TC]: two k-tiles stacked for DoubleRow).
                    xk, xv, xr8 = [], [], []
                    for ig2 in range(G // 2):
                        t8 = dxp.tile([P, 2, TC], FP8, tag=f"xr8{ig2}",
                                      name=f"xr8_{ig2}")
                        xr8.append(t8)
                    for ig in range(G):
                        cur = xw[:, ig, 1:1 + TC]
                        sh = xw[:, ig, 0:TC]
                        dx = dxp.tile([P, TC], BF16, tag=f"dx{ig}")
                        nc.gpsimd.tensor_sub(dx[:], cur, sh)
                        for nm, tmf, lst in (("xk", tmkf, xk),
                                             ("xv", tmvf, xv),
                                             ("xr", tmrf, None)):
                            t1 = t1p.tile([P, TC], BF16, tag=f"t1{nm}")
                            nc.gpsimd.tensor_mul(t1[:], dx[:], tmf[:, ig, :])
                            t = mixp.tile([P, TC], MMDT, tag=f"{nm}{ig}")
                            nc.gpsimd.tensor_add(t[:], t1[:], sh)
                            if lst is None:
                                # quantize the r-mix to fp8 on DVE: ACT is
                                # the co-bottleneck (92% busy — table loads
                                # make its ops ~1.7x pricier than DVE's)
                                nc.vector.tensor_copy(
                                    xr8[ig // 2][:, ig % 2, :], t[:])
                            else:
                                lst.append(t)

                    numout = scano.tile([P, G, TC + 1], F32, tag="numout")
                    denout = scano.tile([P, G, TC + 1], F32, tag="denout")

                    # ---- pass KV: k & v projections + WKV scans ----
                    wsr = wsrp.tile([P, G, TC], MMDT, tag="wsr")
                    for jg in range(G):
                        jsl = bass.ts(jg, P)
                        kps = pskvr.tile([P, TC], F32, tag="kps")
                        for ig in range(G):
                            nc.tensor.matmul(kps[:], wk[:, ig, jsl], xk[ig][:],
                                             start=(ig == 0), stop=(ig == G - 1))
                        vps = pskvr.tile([P, TC], F32, tag="vps")
                        for ig in range(G):
                            nc.tensor.matmul(vps[:], wv[:, ig, jsl], xv[ig][:],
                                             start=(ig == 0), stop=(ig == G - 1))
                        ekap = ekbuf[:, jg, 1:TC + 1]
                        nc.scalar.activation(ekap, kps[:], ACT_F.Exp)
                        nc.vector.tensor_mul(ekvbuf[:, jg, 1:TC + 1], ekap, vps[:])

                        if c == 0:
                            init_n = ln_sb[:, jg:jg + 1]
                            init_d = ld_sb[:, jg:jg + 1]
                        else:
                            init_n = numout_prev[:, jg, TC:TC + 1]
                            init_d = denout_prev[:, jg, TC:TC + 1]
                        nc.vector.tensor_tensor_scan(
                            numout[:, jg, :], scanmul[:, jg, :], ekvbuf[:, jg, :],
                            init_n, ALU.mult, ALU.add)
                        nc.vector.tensor_tensor_scan(
                            denout[:, jg, :], scanmul[:, jg, :], ekbuf[:, jg, :],
                            init_d, ALU.mult, ALU.add)

                        # euk = ek*etf, so:
                        # numer = num_{t-1} + etf*ekv ; denom = den_{t-1} + etf*ek
                        numer = work.tile([P, TC], F32, tag="numer")
                        denom = work.tile([P, TC], F32, tag="denom")
                        etfs = etf_sb[:, jg:jg + 1]
                        nc.vector.scalar_tensor_tensor(
                            numer[:], ekvbuf[:, jg, 1:TC + 1], etfs,
                            numout[:, jg, 0:TC], ALU.mult, ALU.add)
                        nc.vector.scalar_tensor_tensor(
                            denom[:], ekap, etfs,
                            denout[:, jg, 0:TC], ALU.mult, ALU.add)
                        rden = work.tile([P, TC], F32, tag="rden")
                        nc.vector.reciprocal(rden[:], denom[:])
                        nc.gpsimd.tensor_mul(wsr[:, jg], numer[:], rden[:])

                    numout_prev, denout_prev = numout, denout

                    # ---- pass R: fp8 DoubleRow r projections (2 k-tiles per
                    # matmul, 2x PE rate), sigmoid folds the 1/RSCALE back in,
                    # then wsr *= sigmoid(r) ----
                    for jg in range(G):
                        jsl = bass.ts(jg, P)
                        rps = pskvr.tile([P, TC], F32, tag="rps")
                        for ig2 in range(G // 2):
                            nc.tensor.matmul(
                                rps[:], wr[:, 2 * ig2:2 * ig2 + 2, jsl],
                                xr8[ig2][:], start=(ig2 == 0),
                                stop=(ig2 == G // 2 - 1), perf_mode=PM.DoubleRow)
                        sr = work.tile([P, TC], F32, tag="sr")
                        nc.scalar.activation(sr[:], rps[:], ACT_F.Sigmoid,
                                             scale=1.0 / RSCALE)
                        nc.gpsimd.tensor_mul(wsr[:, jg], wsr[:, jg], sr[:])

                    # ---- pass O for the PREVIOUS chunk (software pipeline:
                    # wsr of c-1 is long since ready, so the PE never stalls
                    # on this chunk's gate tail) ----
                    if wsr_prev is not None:
                        emit_opass(wsr_prev, t0_prev)
                    wsr_prev, t0_prev = wsr, t0

                emit_opass(wsr_prev, t0_prev)

            for _ in range(reps or 1):
                emit_all()

    _split_waits(nc, 1)
    return nc


_NC_CACHE = None


def _get_nc():
    global _NC_CACHE
    if _NC_CACHE is None:
        _NC_CACHE = _build_nc()
    return _NC_CACHE


def _pg(v):
    """(D,) channel vector -> [P, G] with channel d = g*128 + p."""
    return np.ascontiguousarray(np.asarray(v, np.float32).reshape(G, P).T)


def _wt(w, dt=None, scale=1.0):
    """W (D_out, D_in) -> W.T tiled [P, G, D_out] (contraction on partitions)."""
    wt = np.asarray(w, np.float32).T * scale  # (D_in, D_out)
    out = np.ascontiguousarray(wt.reshape(G, P, D).transpose(1, 0, 2))
    return out.astype(mybir.dt.np(dt or MMDT))


def kernel(x, last_x, last_num, last_den, time_decay, time_first,
           time_mix_k, time_mix_v, time_mix_r, Wk, Wv, Wr, Wo):
    x = np.asarray(x, np.float32)
    last_x = np.asarray(last_x, np.float32)
    last_num = np.asarray(last_num, np.float32)
    last_den = np.asarray(last_den, np.float32)

    ew = _pg(np.exp(-np.exp(np.asarray(time_decay, np.float64))))
    etf = _pg(np.exp(np.asarray(time_first, np.float64)))
    tmk = _pg(np.asarray(time_mix_k).reshape(-1))
    tmv = _pg(np.asarray(time_mix_v).reshape(-1))
    tmr = _pg(np.asarray(time_mix_r).reshape(-1))
    wkT, wvT, woT = _wt(Wk), _wt(Wv), _wt(Wo)
    wrT = _wt(Wr, dt=FP8, scale=RSCALE)
    bf = mybir.dt.np(BF16)

    in_maps = []
    for b in range(B):
        xs = np.concatenate([last_x[b], x[b]], axis=0)      # (T+1, D)
        xTb = np.ascontiguousarray(
            xs.T.reshape(G, P, T + 1).transpose(1, 0, 2)).astype(bf)
        in_maps.append({
            "xT": xTb,
            "WkT": wkT, "WvT": wvT, "WrT": wrT, "WoT": woT,
            "ew": ew, "etf": etf, "tmk": tmk, "tmv": tmv, "tmr": tmr,
            "lnum": _pg(last_num[b, 0]), "lden": _pg(last_den[b, 0]),
        })

    global _last_in_maps
    _last_in_maps = in_maps
    nc = _get_nc()
    res = run_bass_kernel_spmd(nc, in_maps, list(range(B)))
    return np.stack([res.results[b]["out"] for b in range(B)], axis=0)


_last_in_maps = None
